# revision 1
# baseline (speedup 1.0000x reference)
"""Single-head attention (N=8192, D=128) on 8 Trainium2 NeuronCores.

Sharding: sequence-parallel over query rows.  Core c receives X rolled by
-c*1024 rows, so its 1024 query rows are rows 0..1023 of its input; the key
set is the full (permuted) X, and softmax sums are permutation-invariant.
The same SPMD program runs on all 8 cores; the host concatenates outputs.

Per-core data flow (everything contracts over the SBUF partition dim):
  X -> bf16 -> DRAM -> xbar transpose DMA -> X^T [d, m] in SBUF
  XWT[b, n] = sum_a W[a,b] X^T[a,n]      (lhsT=W, rhs=X^T cols 0..1023)
  per m-chunk (64 chunks of 128 keys):
    sT[m, n]  = sum_b X^T[b,m] XWT[b,n]  (lhsT=X^T chunk, rhs=XWT)
    XV[m, e]  = sum_a X^T[a,m] V[a,e]    (same stationary operand)
    E[m, n]   = exp(sT / sqrt(D))        (ACT, PSUM->SBUF, bf16)
    out[n,e|d] += E[:, nsub]^T @ [XV | 1]  (ones col accumulates the
                                            softmax denominator for free)
  out[n, :128] * recip(out[n, 128]) -> DRAM
"""

import numpy as np
from contextlib import ExitStack

import concourse.bass as bass
import concourse.bacc as bacc
import concourse.mybir as mybir
import concourse.tile as tile
from concourse.bass_utils import run_bass_kernel_spmd

N, D = 8192, 128
NCORES = 8
R = N // NCORES            # query rows per core
NCHUNK = N // 128          # key chunks of 128
NSUB = R // 128            # query subchunks of 128
GRP = 2048                 # transpose-pipeline row group
SCALE = 1.0 / float(np.sqrt(D))

f32 = mybir.dt.float32
bf16 = mybir.dt.float16  # fp16: 11-bit mantissa, same PE/DMA-transpose speed as bf16
Exp = mybir.ActivationFunctionType.Exp


def _build():
    nc = bacc.Bacc(
        "TRN2", target_bir_lowering=False, debug=False, num_devices=NCORES
    )
    x_d = nc.dram_tensor("X", [N, D], f32, kind="ExternalInput").ap()
    w_d = nc.dram_tensor("W", [1, D, D], f32, kind="ExternalInput").ap()
    v_d = nc.dram_tensor("V", [1, D, D], f32, kind="ExternalInput").ap()
    out_d = nc.dram_tensor("out", [R, D], f32, kind="ExternalOutput").ap()
    scr_d = nc.dram_tensor("xscratch", [N, D], bf16).ap()

    with tile.TileContext(nc) as tc:
        with ExitStack() as ctx:
            const = ctx.enter_context(tc.tile_pool(name="const", bufs=1))
            epool = ctx.enter_context(tc.tile_pool(name="exp", bufs=3))
            opool = ctx.enter_context(tc.tile_pool(name="outp", bufs=1))
            spsum = ctx.enter_context(
                tc.tile_pool(name="spsum", bufs=2, space="PSUM")
            )
            vpsum = ctx.enter_context(
                tc.tile_pool(name="vpsum", bufs=1, space="PSUM")
            )
            ppsum = ctx.enter_context(
                tc.tile_pool(name="ppsum", bufs=1, space="PSUM")
            )

            xt = const.tile([D, N], bf16)                 # X^T [a, m]
            xv = const.tile([128, NCHUNK * (D + 1)], bf16)  # [XV | 1] chunks
            xv3 = xv[:].rearrange("p (c q) -> p c q", q=D + 1)

            # X -> fp16 DRAM scratch (cast DMA) -> xbar-transposed into xt.
            # The first (small) group's cast is issued before anything else
            # on the Pool engine so the transpose pipeline starts ASAP.
            w_sb = const.tile([D, D], bf16)
            v_sb = const.tile([D, D], bf16)
            row0 = 0
            for grp in (1024, 1024, 2048, 4096):
                rows = slice(row0, row0 + grp)
                nc.gpsimd.dma_start(scr_d[rows, :], x_d[rows, :])
                if row0 == 0:
                    nc.gpsimd.dma_start(w_sb[:], w_d[0])
                    nc.gpsimd.dma_start(v_sb[:], v_d[0])
                    nc.vector.memset(xv3[:, :, D], 1.0)
                nc.sync.dma_start_transpose(xt[:, rows], scr_d[rows, :])
                row0 += grp

            # XWT[b, n] for this core's query columns
            xwt_ps = spsum.tile([128, R], f32, tag="s")
            nc.tensor.matmul(
                xwt_ps[:, 0:512], w_sb[:], xt[:, 0:512], start=True, stop=True
            )
            nc.tensor.matmul(
                xwt_ps[:, 512:R], w_sb[:], xt[:, 512:R], start=True, stop=True
            )
            xwt = const.tile([D, R], bf16)
            nc.vector.tensor_copy(xwt[:], xwt_ps[:])

            # persistent PV accumulators: 8 slices of width 129 packed 3/3/2.
            # Slot stride 132 keeps every slice 16B-aligned so accumulating
            # matmuls never share a PSUM has_written granule with a
            # neighboring slice (a shared granule loses the first
            # accumulation step when the neighbor's start=True clears it).
            PVS = 132
            pv_tiles = [
                ppsum.tile([128, w * PVS], f32, tag=f"pv{i}", name=f"pv{i}")
                for i, w in enumerate((3, 3, 2))
            ]

            def pv_slice(j):
                return pv_tiles[j // 3], (j % 3) * PVS

            # A start=True matmul clears has_written for its whole PSUM
            # bank, so per-slice start flags would wipe sibling slices'
            # first accumulation.  Instead, zero each PV bank once with a
            # dummy start=True matmul (zero weights), then accumulate every
            # PV matmul with start=False.
            zeros_bf = const.tile([128, 128], bf16)
            nc.vector.memset(zeros_bf[:], 0.0)
            for t in pv_tiles:
                nc.tensor.matmul(
                    t[:],
                    zeros_bf[:],
                    xt[:, 0 : t.shape[1]],
                    start=True,
                    stop=False,
                    skip_group_check=True,
                )

            for mc in range(NCHUNK):
                lhs = xt[:, mc * 128 : (mc + 1) * 128]
                s_ps = spsum.tile([128, R], f32, tag="s")
                nc.tensor.matmul(
                    s_ps[:, 0:512], lhs, xwt[:, 0:512], start=True, stop=True
                )
                nc.tensor.matmul(
                    s_ps[:, 512:R], lhs, xwt[:, 512:R], start=True, stop=True
                )
                xv_ps = vpsum.tile([128, D], f32, tag="xv")
                nc.tensor.matmul(xv_ps[:], lhs, v_sb[:], start=True, stop=True)
                nc.vector.tensor_copy(xv3[:, mc, 0:D], xv_ps[:])

                e_t = epool.tile([128, R], bf16, tag="e")
                nc.scalar.activation(e_t[:], s_ps[:], Exp, scale=SCALE)

                for j in range(NSUB):
                    pvt, off = pv_slice(j)
                    nc.tensor.matmul(
                        pvt[:, off : off + D + 1],
                        e_t[:, j * 128 : (j + 1) * 128],
                        xv3[:, mc, :],
                        start=False,
                        stop=(mc == NCHUNK - 1),
                        skip_group_check=True,
                    )

            rec = const.tile([128, NSUB], f32)
            o_all = opool.tile([128, NSUB * D], f32)
            for j in range(NSUB):
                pvt, off = pv_slice(j)
                nc.vector.reciprocal(
                    rec[:, j : j + 1], pvt[:, off + D : off + D + 1]
                )
                nc.vector.tensor_scalar_mul(
                    o_all[:, j * D : (j + 1) * D],
                    pvt[:, off : off + D],
                    rec[:, j : j + 1],
                )
            nc.sync.dma_start(
                out_d.rearrange("(j p) e -> p j e", p=128),
                o_all[:].rearrange("p (j e) -> p j e", e=D),
            )

    nc.compile()
    return nc


_cached_nc = None


def _get_nc():
    global _cached_nc
    if _cached_nc is None:
        _cached_nc = _build()
    return _cached_nc


def run(X, W, V, trace=False):
    import time

    X = np.ascontiguousarray(np.asarray(X, dtype=np.float32))
    W = np.ascontiguousarray(np.asarray(W, dtype=np.float32))
    V = np.ascontiguousarray(np.asarray(V, dtype=np.float32))
    nc = _get_nc()
    in_maps = [
        {"X": np.roll(X, -c * R, axis=0), "W": W, "V": V}
        for c in range(NCORES)
    ]
    t0 = time.time()
    res = run_bass_kernel_spmd(nc, in_maps, list(range(NCORES)))
    t1 = time.time()
    out = np.concatenate(
        [res.results[c]["out"] for c in range(NCORES)], axis=0
    )
    return out, {"res": res, "spmd_wall_s": t1 - t0}


def timeline_ns():
    """Cost-model (TimelineSim) estimate of one core's execution, in ns."""
    from concourse.timeline_sim import TimelineSim

    nc = _get_nc()
    tl = TimelineSim(nc)
    tl.simulate()
    return float(tl.time)


def kernel(X, W, V):
    out, _ = run(X, W, V)
    return out

